# revision 2
# baseline (speedup 1.0000x reference)
"""Trainium2 Bass kernel for nn_Attention_85074712199827, v6.

    h_proj  = hidden[0] @ W_attn[:, :1024].T            (host, 67 MFLOP)
    e_proj  = encoder_outputs @ W_attn[:, 1024:].T      (device PE, 137 GFLOP)
    energy  = tanh(e_proj + h_proj[:, None, :] + b_attn)
    att     = energy @ v
    out     = softmax(att, axis=1)          # [32, 2048] float32

Data-parallel over batch across 8 NeuronCores (4 rows per core).

Host pre-layout: enc cast bf16, transposed h-major, and packed
per-unit contiguous (one fat-descriptor DMA per s-block unit); We
pre-chunked by output o-block; h_proj+b_attn folded into the tanh
bias table; v and a ones-column shipped for the v-dot.

Device: per unit, 8x8 accumulating bf16 matmuls -> tanh (scalar,
bias per-partition) -> v-scale (vector) + add tree -> ones-vector
partition-sum matmul -> Exp with accum_out (online softmax without
max subtraction; logits are tanh-bounded). All input DMAs ride one
HWDGE ring in priority order; outputs go per-batch-row.
"""

from contextlib import ExitStack

import ml_dtypes
import numpy as np

import concourse.bass as bass
import concourse.tile as tile
from concourse import bacc, mybir

F32 = mybir.dt.float32
BF16 = mybir.dt.bfloat16
AF = mybir.ActivationFunctionType
ADD = mybir.AluOpType.add
P = 128
SB = 512


def unit_list(b_loc=4, s=2048, split_first=2, split_last=2):
    units = []
    for b in range(b_loc):
        blocks = [(i * SB, SB) for i in range(s // SB)]
        if b == 0 and split_first > 1:
            st, ln = blocks[0]
            sub = ln // split_first
            blocks = [(st + i * sub, sub) for i in range(split_first)] + blocks[1:]
        if b == b_loc - 1 and split_last > 1:
            st, ln = blocks[-1]
            sub = ln // split_last
            blocks = blocks[:-1] + [(st + i * sub, sub) for i in range(split_last)]
        for ci, (st, ln) in enumerate(blocks):
            units.append((b, st, ln, ci))
    return units


def build_nc(b_loc=4, s=2048, h=1024, n_cores=8,
             warmup_mm=14, warm_n=256,
             split_first=2, split_last=2,
             eT_bufs=5, pe_bufs=6, en_bufs=3, ys_bufs=2, pa_bufs=2,
             wt_head=3):
    n_hc = h // P
    n_ot = h // P

    units = unit_list(b_loc, s, split_first, split_last)
    n_cols = max(u[3] for u in units) + 1

    nc = bacc.Bacc("TRN2", target_bir_lowering=False, debug=False,
                   num_devices=n_cores)

    total = sum(P * n_hc * ln for (_, _, ln, _) in units)
    encU = nc.dram_tensor("encU", [total], BF16, kind="ExternalInput").ap()
    unit_off = {}
    off = 0
    for (b, s0, ln, _) in units:
        unit_off[(b, s0)] = off
        off += P * n_hc * ln

    wtE = nc.dram_tensor("wtE", [n_ot, P, n_hc, P], BF16,
                         kind="ExternalInput").ap()
    hb = nc.dram_tensor("hb", [P, n_ot, b_loc], F32, kind="ExternalInput").ap()
    vt = nc.dram_tensor("vt", [P, n_ot], F32, kind="ExternalInput").ap()
    vtb = nc.dram_tensor("vtb", [P, n_ot], BF16, kind="ExternalInput").ap()
    ones = nc.dram_tensor("ones", [P, 1], BF16, kind="ExternalInput").ap()
    out = nc.dram_tensor("out", [b_loc, s], F32, kind="ExternalOutput").ap()

    with tile.TileContext(nc) as tc, ExitStack() as ctx:
        const = ctx.enter_context(tc.tile_pool(name="const", bufs=1))
        pe_p = ctx.enter_context(tc.tile_pool(name="pe", bufs=pe_bufs, space="PSUM"))
        pa_p = ctx.enter_context(tc.tile_pool(name="pa", bufs=pa_bufs, space="PSUM"))

        # ---- warmup matmuls (through the pe pool; no extra PSUM
        # banks): lift the HAM clock gate while first DMAs fly ----
        wz = const.tile([P, SB], BF16)
        nc.vector.memset(wz[:], 0)
        for _ in range(warmup_mm):
            pw = pe_p.tile([P, SB], F32, name="pe")
            nc.tensor.matmul(pw[:, :warm_n], wz[:, :P], wz[:, :warm_n],
                             start=True, stop=True)

        # ---- small constants on the second HWDGE ring (scalar) ----
        hbt = const.tile([P, n_ot, b_loc], F32)
        nc.scalar.dma_start(hbt[:], hb)
        vtt = const.tile([P, n_ot], F32)
        nc.scalar.dma_start(vtt[:], vt)
        vttb = const.tile([P, n_ot], BF16)
        nc.scalar.dma_start(vttb[:], vtb)
        onest = const.tile([P, 1], BF16)
        nc.scalar.dma_start(onest[:], ones)

        # ---- pipeline pools ----
        eT_p = ctx.enter_context(tc.tile_pool(name="eT", bufs=eT_bufs))
        eTs_p = ctx.enter_context(tc.tile_pool(name="eTs", bufs=3))
        en_p = ctx.enter_context(tc.tile_pool(name="en", bufs=en_bufs))
        ys_p = ctx.enter_context(tc.tile_pool(name="ys", bufs=ys_bufs))

        wt_t = [None] * n_ot

        def load_wt(ot):
            w = const.tile([P, n_hc, P], BF16, name=f"wt{ot}")
            nc.sync.dma_start(w[:], wtE[ot])
            wt_t[ot] = w

        def load_unit(u):
            b, s0, ln, _ = u
            pool = eTs_p if ln < SB else eT_p
            eT = pool.tile([P, n_hc, ln], BF16, name="eT")
            o0 = unit_off[(b, s0)]
            src_ap = encU[o0:o0 + P * n_hc * ln].rearrange(
                "(p hc sj) -> p hc sj", p=P, hc=n_hc)
            nc.sync.dma_start(eT[:], src_ap)
            return eT

        e_rows = [const.tile([1, s], F32, name=f"e{b}") for b in range(b_loc)]
        spart = [const.tile([1, n_cols], F32, name=f"sp{b}")
                 for b in range(b_loc)]
        o_rows = [const.tile([1, s], F32, name=f"o{b}") for b in range(b_loc)]
        rinv = [const.tile([1, 1], F32, name=f"ri{b}") for b in range(b_loc)]

        TREE = [(0, 1), (2, 3), (4, 5), (6, 7), (0, 2), (4, 6), (0, 4)]

        def compute_unit(u, eT, pe_vdot=False):
            b, s0, ln, col = u
            sl = slice(s0, s0 + ln)
            pa = pa_p.tile([1, SB], F32, name="pa")[:, :ln]
            if not pe_vdot:
                ys = ys_p.tile([P, n_ot, SB], BF16, name="ys")[:, :, :ln]
            pending = None
            for ot in range(n_ot):
                pe = pe_p.tile([P, SB], F32, name="pe")[:, :ln]
                for hc in range(n_hc):
                    nc.tensor.matmul(
                        pe, wt_t[ot][:, hc, :], eT[:, hc, :],
                        start=(hc == 0), stop=(hc == n_hc - 1))
                eng = en_p.tile([P, SB], BF16, name="eng")[:, :ln]
                nc.scalar.activation(eng, pe, AF.Tanh, bias=hbt[:, ot, b:b + 1])
                if pe_vdot:
                    if pending is not None:
                        pot, peng = pending
                        nc.tensor.matmul(
                            pa, vttb[:, pot:pot + 1], peng,
                            start=(pot == 0), stop=False, skip_group_check=True)
                    pending = (ot, eng)
                else:
                    nc.vector.tensor_scalar_mul(
                        ys[:, ot, :], eng, vtt[:, ot:ot + 1])
            if pe_vdot:
                pot, peng = pending
                nc.tensor.matmul(
                    pa, vttb[:, pot:pot + 1], peng,
                    start=(pot == 0), stop=True, skip_group_check=True)
            else:
                for i, j in TREE:
                    nc.vector.tensor_tensor(
                        ys[:, i, :], ys[:, i, :], ys[:, j, :], ADD)
                nc.tensor.matmul(pa, onest[:], ys[:, 0, :], start=True, stop=True)
            nc.scalar.activation(
                e_rows[b][0:1, sl], pa, AF.Exp,
                accum_out=spart[b][0:1, col:col + 1])

        def finalize_b(b, blocks):
            ssum = const.tile([1, 1], F32, name=f"ss{b}")
            nc.vector.tensor_reduce(
                ssum[:], spart[b][0:1, :len(blocks)], mybir.AxisListType.X, ADD)
            nc.vector.reciprocal(rinv[b][:], ssum[:])
            for ci, (st, ln) in enumerate(blocks):
                sl = slice(st, st + ln)
                if ci % 2 == 0:
                    nc.scalar.activation(
                        o_rows[b][0:1, sl], e_rows[b][0:1, sl], AF.Copy,
                        scale=rinv[b][:])
                else:
                    nc.vector.tensor_scalar_mul(
                        o_rows[b][0:1, sl], e_rows[b][0:1, sl], rinv[b][:])
            nc.sync.dma_start(out[b:b + 1, :], o_rows[b][:])

        # ---- priority-ordered single-ring input stream ----
        eTs = {}
        eTs[0] = load_unit(units[0])
        for ot in range(wt_head):
            load_wt(ot)
        eTs[1] = load_unit(units[1])
        for ot in range(wt_head, n_ot):
            load_wt(ot)
        eTs[2] = load_unit(units[2])

        by_b = {}
        n_by_b = {}
        for u in units:
            n_by_b[u[0]] = n_by_b.get(u[0], 0) + 1
        for ui, u in enumerate(units):
            if ui + 3 < len(units):
                eTs[ui + 3] = load_unit(units[ui + 3])
            compute_unit(u, eTs.pop(ui), pe_vdot=(ui == len(units) - 1))
            b, s0, ln, col = u
            by_b.setdefault(b, []).append((s0, ln))
            if col == n_by_b[b] - 1:
                finalize_b(b, by_b[b])

    nc.compile()
    return nc


def make_in_maps(hidden, encoder_outputs, W_attn, b_attn, v, n_cores=8):
    BF = ml_dtypes.bfloat16
    hidden = np.asarray(hidden, dtype=np.float32)
    encoder_outputs = np.asarray(encoder_outputs, dtype=np.float32)
    W_attn = np.asarray(W_attn, dtype=np.float32)
    b_attn = np.asarray(b_attn, dtype=np.float32)
    v = np.asarray(v, dtype=np.float32)

    b, s, h = encoder_outputs.shape
    b_loc = b // n_cores
    n_hc = h // P

    Wh = W_attn[:, :h]
    We = W_attn[:, h:]
    wtE = np.ascontiguousarray(
        We.T.reshape(n_hc, P, h // P, P).transpose(2, 1, 0, 3).astype(BF))
    vt = np.ascontiguousarray(v.reshape(h // P, P).T.astype(np.float32))
    vtb = np.ascontiguousarray(v.reshape(h // P, P).T.astype(BF))
    ones = np.ones((P, 1), dtype=BF)

    h_proj = hidden[0] @ Wh.T + b_attn          # [B, h] f32

    units = unit_list(b_loc, s)
    in_maps = []
    for i in range(n_cores):
        bsl = slice(b_loc * i, b_loc * (i + 1))
        e = encoder_outputs[bsl].astype(BF)     # [b_loc, s, h]
        e = e.transpose(0, 2, 1)                # [b_loc, h, s]
        e = e.reshape(b_loc, n_hc, P, s)        # [b, hc, p, s]
        e = np.ascontiguousarray(e.transpose(0, 2, 1, 3))   # [b, p, hc, s]
        blocks = [np.ascontiguousarray(e[b][:, :, s0:s0 + ln]).reshape(-1)
                  for (b, s0, ln, _) in units]
        encU = np.concatenate(blocks)
        hp = h_proj[bsl]
        hbt = np.ascontiguousarray(
            hp.T.reshape(n_hc, P, b_loc).transpose(1, 0, 2))
        in_maps.append({
            "encU": encU,
            "wtE": wtE,
            "hb": hbt,
            "vt": vt,
            "vtb": vtb,
            "ones": ones,
        })
    return in_maps


_NC_CACHE = {}


def _get_nc():
    if "nc" not in _NC_CACHE:
        _NC_CACHE["nc"] = build_nc(b_loc=4, s=2048, h=1024, n_cores=8)
    return _NC_CACHE["nc"]


def kernel(hidden, encoder_outputs, W_attn, b_attn, v):
    from concourse.bass_utils import run_bass_kernel_spmd

    nc = _get_nc()
    in_maps = make_in_maps(hidden, encoder_outputs, W_attn, b_attn, v,
                           n_cores=8)
    res = run_bass_kernel_spmd(nc, in_maps, core_ids=list(range(8)))
    out = np.concatenate([np.asarray(res.results[i]["out"])
                          for i in range(8)], axis=0)
    return out.astype(np.float32)


# revision 3
# speedup vs baseline: 1.0036x; 1.0036x over previous
"""Trainium2 Bass kernel for nn_Attention_85074712199827, v6.

    h_proj  = hidden[0] @ W_attn[:, :1024].T            (host, 67 MFLOP)
    e_proj  = encoder_outputs @ W_attn[:, 1024:].T      (device PE, 137 GFLOP)
    energy  = tanh(e_proj + h_proj[:, None, :] + b_attn)
    att     = energy @ v
    out     = softmax(att, axis=1)          # [32, 2048] float32

Data-parallel over batch across 8 NeuronCores (4 rows per core).

Host pre-layout: enc cast bf16, transposed h-major, and packed
per-unit contiguous (one fat-descriptor DMA per s-block unit); We
pre-chunked by output o-block; h_proj+b_attn folded into the tanh
bias table; v and a ones-column shipped for the v-dot.

Device: per unit, 8x8 accumulating bf16 matmuls -> tanh (scalar,
bias per-partition) -> v-scale (vector) + add tree -> ones-vector
partition-sum matmul -> Exp with accum_out (online softmax without
max subtraction; logits are tanh-bounded). All input DMAs ride one
HWDGE ring in priority order; outputs go per-batch-row.
"""

from contextlib import ExitStack

import ml_dtypes
import numpy as np

import concourse.bass as bass
import concourse.tile as tile
from concourse import bacc, mybir

F32 = mybir.dt.float32
BF16 = mybir.dt.bfloat16
AF = mybir.ActivationFunctionType
ADD = mybir.AluOpType.add
P = 128
SB = 512


def unit_list(b_loc=4, s=2048, split_first=2, split_last=2):
    units = []
    for b in range(b_loc):
        blocks = [(i * SB, SB) for i in range(s // SB)]
        if b == 0 and split_first > 1:
            st, ln = blocks[0]
            sub = ln // split_first
            blocks = [(st + i * sub, sub) for i in range(split_first)] + blocks[1:]
        if b == b_loc - 1 and split_last > 1:
            st, ln = blocks[-1]
            sub = ln // split_last
            blocks = blocks[:-1] + [(st + i * sub, sub) for i in range(split_last)]
        for ci, (st, ln) in enumerate(blocks):
            units.append((b, st, ln, ci))
    return units


def build_nc(b_loc=4, s=2048, h=1024, n_cores=8,
             warmup_mm=14, warm_n=256,
             split_first=2, split_last=2,
             eT_bufs=5, pe_bufs=6, en_bufs=3, ys_bufs=2, pa_bufs=2,
             wt_head=3):
    n_hc = h // P
    n_ot = h // P

    units = unit_list(b_loc, s, split_first, split_last)
    n_cols = max(u[3] for u in units) + 1

    nc = bacc.Bacc("TRN2", target_bir_lowering=False, debug=False,
                   num_devices=n_cores)

    total = sum(P * n_hc * ln for (_, _, ln, _) in units)
    encU = nc.dram_tensor("encU", [total], BF16, kind="ExternalInput").ap()
    unit_off = {}
    off = 0
    for (b, s0, ln, _) in units:
        unit_off[(b, s0)] = off
        off += P * n_hc * ln

    wtE = nc.dram_tensor("wtE", [n_ot, P, n_hc, P], BF16,
                         kind="ExternalInput").ap()
    hb = nc.dram_tensor("hb", [P, n_ot, b_loc], F32, kind="ExternalInput").ap()
    vt = nc.dram_tensor("vt", [P, n_ot], F32, kind="ExternalInput").ap()
    vtb = nc.dram_tensor("vtb", [P, n_ot], BF16, kind="ExternalInput").ap()
    ones = nc.dram_tensor("ones", [P, 1], BF16, kind="ExternalInput").ap()
    out = nc.dram_tensor("out", [b_loc, s], F32, kind="ExternalOutput").ap()

    with tile.TileContext(nc) as tc, ExitStack() as ctx:
        const = ctx.enter_context(tc.tile_pool(name="const", bufs=1))
        pe_p = ctx.enter_context(tc.tile_pool(name="pe", bufs=pe_bufs, space="PSUM"))
        pa_p = ctx.enter_context(tc.tile_pool(name="pa", bufs=pa_bufs, space="PSUM"))

        # ---- warmup matmuls (through the pe pool; no extra PSUM
        # banks): lift the HAM clock gate while first DMAs fly ----
        wz = const.tile([P, SB], BF16)
        nc.vector.memset(wz[:], 0)
        for _ in range(warmup_mm):
            pw = pe_p.tile([P, SB], F32, name="pe")
            nc.tensor.matmul(pw[:, :warm_n], wz[:, :P], wz[:, :warm_n],
                             start=True, stop=True)

        # ---- small constants on the second HWDGE ring (scalar) ----
        hbt = const.tile([P, n_ot, b_loc], F32)
        nc.scalar.dma_start(hbt[:], hb)
        vtt = const.tile([P, n_ot], F32)
        nc.scalar.dma_start(vtt[:], vt)
        vttb = const.tile([P, n_ot], BF16)
        nc.scalar.dma_start(vttb[:], vtb)
        onest = const.tile([P, 1], BF16)
        nc.scalar.dma_start(onest[:], ones)

        # ---- pipeline pools ----
        eT_p = ctx.enter_context(tc.tile_pool(name="eT", bufs=eT_bufs))
        eTs_p = ctx.enter_context(tc.tile_pool(name="eTs", bufs=3))
        en_p = ctx.enter_context(tc.tile_pool(name="en", bufs=en_bufs))
        ys_p = ctx.enter_context(tc.tile_pool(name="ys", bufs=ys_bufs))

        wt_t = [None] * n_ot

        def load_wt(ot):
            w = const.tile([P, n_hc, P], BF16, name=f"wt{ot}")
            nc.sync.dma_start(w[:], wtE[ot])
            wt_t[ot] = w

        def load_unit(u):
            b, s0, ln, _ = u
            pool = eTs_p if ln < SB else eT_p
            eT = pool.tile([P, n_hc, ln], BF16, name="eT")
            o0 = unit_off[(b, s0)]
            src_ap = encU[o0:o0 + P * n_hc * ln].rearrange(
                "(p hc sj) -> p hc sj", p=P, hc=n_hc)
            nc.sync.dma_start(eT[:], src_ap)
            return eT

        e_rows = [const.tile([1, s], F32, name=f"e{b}") for b in range(b_loc)]
        spart = [const.tile([1, n_cols], F32, name=f"sp{b}")
                 for b in range(b_loc)]
        o_rows = [const.tile([1, s], F32, name=f"o{b}") for b in range(b_loc)]
        rinv = [const.tile([1, 1], F32, name=f"ri{b}") for b in range(b_loc)]

        TREE = [(0, 1), (2, 3), (4, 5), (6, 7), (0, 2), (4, 6), (0, 4)]

        # deferred emitters: unit u's partition-sum MM and exp are
        # emitted mid-unit-u+1 so they never block the PE/scalar FIFO
        # queues while waiting on the vector add-tree
        deferred = []

        def drain_deferred():
            while deferred:
                deferred.pop(0)()

        def compute_unit(u, eT, pe_vdot=False):
            b, s0, ln, col = u
            sl = slice(s0, s0 + ln)
            pa = pa_p.tile([1, SB], F32, name="pa")[:, :ln]
            if not pe_vdot:
                ys = ys_p.tile([P, n_ot, SB], BF16, name="ys")[:, :, :ln]
            pending = None
            for ot in range(n_ot):
                pe = pe_p.tile([P, SB], F32, name="pe")[:, :ln]
                for hc in range(n_hc):
                    nc.tensor.matmul(
                        pe, wt_t[ot][:, hc, :], eT[:, hc, :],
                        start=(hc == 0), stop=(hc == n_hc - 1))
                eng = en_p.tile([P, SB], BF16, name="eng")[:, :ln]
                nc.scalar.activation(eng, pe, AF.Tanh, bias=hbt[:, ot, b:b + 1])
                if pe_vdot:
                    if pending is not None:
                        pot, peng = pending
                        nc.tensor.matmul(
                            pa, vttb[:, pot:pot + 1], peng,
                            start=(pot == 0), stop=False, skip_group_check=True)
                    pending = (ot, eng)
                else:
                    nc.vector.tensor_scalar_mul(
                        ys[:, ot, :], eng, vtt[:, ot:ot + 1])
                if ot in (1, 2) and deferred:
                    deferred.pop(0)()
            if pe_vdot:
                drain_deferred()
                pot, peng = pending
                nc.tensor.matmul(
                    pa, vttb[:, pot:pot + 1], peng,
                    start=(pot == 0), stop=True, skip_group_check=True)
                nc.scalar.activation(
                    e_rows[b][0:1, sl], pa, AF.Exp,
                    accum_out=spart[b][0:1, col:col + 1])
                return
            for i, j in TREE:
                nc.vector.tensor_tensor(
                    ys[:, i, :], ys[:, i, :], ys[:, j, :], ADD)

            def emit_pa(pa=pa, ys=ys):
                nc.tensor.matmul(pa, onest[:], ys[:, 0, :],
                                 start=True, stop=True)

            def emit_exp(pa=pa, b=b, sl=sl, col=col):
                nc.scalar.activation(
                    e_rows[b][0:1, sl], pa, AF.Exp,
                    accum_out=spart[b][0:1, col:col + 1])

            deferred.append(emit_pa)
            deferred.append(emit_exp)

        def finalize_b(b, blocks):
            ssum = const.tile([1, 1], F32, name=f"ss{b}")
            nc.vector.tensor_reduce(
                ssum[:], spart[b][0:1, :len(blocks)], mybir.AxisListType.X, ADD)
            nc.vector.reciprocal(rinv[b][:], ssum[:])
            for ci, (st, ln) in enumerate(blocks):
                sl = slice(st, st + ln)
                if ci % 2 == 0:
                    nc.scalar.activation(
                        o_rows[b][0:1, sl], e_rows[b][0:1, sl], AF.Copy,
                        scale=rinv[b][:])
                else:
                    nc.vector.tensor_scalar_mul(
                        o_rows[b][0:1, sl], e_rows[b][0:1, sl], rinv[b][:])
            nc.sync.dma_start(out[b:b + 1, :], o_rows[b][:])

        # ---- priority-ordered single-ring input stream ----
        eTs = {}
        eTs[0] = load_unit(units[0])
        for ot in range(wt_head):
            load_wt(ot)
        eTs[1] = load_unit(units[1])
        for ot in range(wt_head, n_ot):
            load_wt(ot)
        eTs[2] = load_unit(units[2])

        by_b = {}
        n_by_b = {}
        for u in units:
            n_by_b[u[0]] = n_by_b.get(u[0], 0) + 1
        for ui, u in enumerate(units):
            if ui + 3 < len(units):
                eTs[ui + 3] = load_unit(units[ui + 3])
            compute_unit(u, eTs.pop(ui), pe_vdot=(ui == len(units) - 1))
            b, s0, ln, col = u
            by_b.setdefault(b, []).append((s0, ln))
            if col == n_by_b[b] - 1:
                drain_deferred()
                finalize_b(b, by_b[b])

    nc.compile()
    return nc


def make_in_maps(hidden, encoder_outputs, W_attn, b_attn, v, n_cores=8):
    BF = ml_dtypes.bfloat16
    hidden = np.asarray(hidden, dtype=np.float32)
    encoder_outputs = np.asarray(encoder_outputs, dtype=np.float32)
    W_attn = np.asarray(W_attn, dtype=np.float32)
    b_attn = np.asarray(b_attn, dtype=np.float32)
    v = np.asarray(v, dtype=np.float32)

    b, s, h = encoder_outputs.shape
    b_loc = b // n_cores
    n_hc = h // P

    Wh = W_attn[:, :h]
    We = W_attn[:, h:]
    wtE = np.ascontiguousarray(
        We.T.reshape(n_hc, P, h // P, P).transpose(2, 1, 0, 3).astype(BF))
    vt = np.ascontiguousarray(v.reshape(h // P, P).T.astype(np.float32))
    vtb = np.ascontiguousarray(v.reshape(h // P, P).T.astype(BF))
    ones = np.ones((P, 1), dtype=BF)

    h_proj = hidden[0] @ Wh.T + b_attn          # [B, h] f32

    units = unit_list(b_loc, s)
    in_maps = []
    for i in range(n_cores):
        bsl = slice(b_loc * i, b_loc * (i + 1))
        e = encoder_outputs[bsl].astype(BF)     # [b_loc, s, h]
        e = e.transpose(0, 2, 1)                # [b_loc, h, s]
        e = e.reshape(b_loc, n_hc, P, s)        # [b, hc, p, s]
        e = np.ascontiguousarray(e.transpose(0, 2, 1, 3))   # [b, p, hc, s]
        blocks = [np.ascontiguousarray(e[b][:, :, s0:s0 + ln]).reshape(-1)
                  for (b, s0, ln, _) in units]
        encU = np.concatenate(blocks)
        hp = h_proj[bsl]
        hbt = np.ascontiguousarray(
            hp.T.reshape(n_hc, P, b_loc).transpose(1, 0, 2))
        in_maps.append({
            "encU": encU,
            "wtE": wtE,
            "hb": hbt,
            "vt": vt,
            "vtb": vtb,
            "ones": ones,
        })
    return in_maps


_NC_CACHE = {}


def _get_nc():
    if "nc" not in _NC_CACHE:
        _NC_CACHE["nc"] = build_nc(b_loc=4, s=2048, h=1024, n_cores=8)
    return _NC_CACHE["nc"]


def kernel(hidden, encoder_outputs, W_attn, b_attn, v):
    from concourse.bass_utils import run_bass_kernel_spmd

    nc = _get_nc()
    in_maps = make_in_maps(hidden, encoder_outputs, W_attn, b_attn, v,
                           n_cores=8)
    res = run_bass_kernel_spmd(nc, in_maps, core_ids=list(range(8)))
    out = np.concatenate([np.asarray(res.results[i]["out"])
                          for i in range(8)], axis=0)
    return out.astype(np.float32)
